# revision 16
# baseline (speedup 1.0000x reference)
"""Trainium2 Bass kernel for nn_Attention (B=2, N=4096, D=1024, 16 heads).

Sharding: 8 cores = 2 (batch) x 4 (head groups of 4 heads, Megatron TP).
Each core computes qkv for its 4 heads, flash-style attention (S^T layout,
softmax denominator via a ones-column folded into the V stationary), and its
partial output projection. The 4 partial projections per batch are summed on
the host during unshard (the TP all-reduce), plus the bias.

v2: software-pipelined instruction emission. The attention j-loop is the
backbone (ACT exp-bound, ~1.06us/iter); everything else (residual QKV
matmul groups, softmax normalization, output projection) is emitted as
background micro-steps interleaved into subsequent j-loops so the PE/ACT
streams never stall at block boundaries. PSUM accumulators are freed at
j-loop end by a single DVE copy to SBUF; the normalize chain (recip ->
gpsimd partition_broadcast -> multiply) runs off the critical path.
"""

from collections import deque

import numpy as np

import concourse.bacc as bacc
import concourse.mybir as mybir
import concourse.tile as tile

B = 2
N = 4096
D = 1024
HL = 4          # heads per core
HD = 64         # head dim
DG = HL * HD    # 256 = per-core d' width
SCALE = HD ** -0.5

FP32 = mybir.dt.float32
BF16 = mybir.dt.bfloat16
MULT = mybir.AluOpType.mult
EXP = mybir.ActivationFunctionType.Exp


def _build(n=N):
    nc = bacc.Bacc("TRN2", target_bir_lowering=False, debug=False)

    xT = nc.declare_dram_parameter("xT", [D, n], BF16, isOutput=False)
    wqT = nc.declare_dram_parameter("wqT", [D, DG], BF16, isOutput=False)
    wkT = nc.declare_dram_parameter("wkT", [D, DG], BF16, isOutput=False)
    wvT = nc.declare_dram_parameter("wvT", [D, DG], BF16, isOutput=False)
    wpT2 = nc.declare_dram_parameter("wpT2", [128, 2, D], BF16, isOutput=False)
    out = nc.declare_dram_parameter("out", [n, D], FP32, isOutput=True)

    DT = D // 128        # 8 contraction tiles for qkv
    NT = n // 128        # key tiles
    QC = min(1024, n)    # qkv prefix group width
    NQC = n // QC
    NC = n // 512        # attention i-chunks

    xT_r = xT.rearrange("(dt p) n -> dt p n", p=128)

    with tile.TileContext(nc) as tc:
        with (
            tc.tile_pool(name="sb", bufs=1) as sb,
            tc.tile_pool(name="wkp", bufs=1) as wkpool,
            tc.tile_pool(name="ps", bufs=1, space="PSUM") as ps,
        ):
            # ---- persistent SBUF tiles ----
            xt = sb.tile([128, DT, n], BF16, tag="xt")
            wq_t = sb.tile([128, DT, DG], BF16, tag="wq")
            wk_t = sb.tile([128, DT, DG], BF16, tag="wk")
            wv_t = sb.tile([128, DT, DG], BF16, tag="wv")
            wp_t = sb.tile([128, 2, D], BF16, tag="wp")
            qt = sb.tile([128, 2, n], BF16, tag="qt")
            kt = sb.tile([128, 2, n], BF16, tag="kt")
            vaug = sb.tile([128, NT, HL, 65], BF16, tag="vaug")
            otn = sb.tile([128, 2, n], BF16, tag="otn")

            # ---- load weights + x^T ----
            wqT_r = wqT.rearrange("(dt p) m -> dt p m", p=128)
            wkT_r = wkT.rearrange("(dt p) m -> dt p m", p=128)
            wvT_r = wvT.rearrange("(dt p) m -> dt p m", p=128)
            for dt_i in range(DT):
                nc.sync.dma_start(wk_t[:, dt_i, :], wkT_r[dt_i, :, :])
            for dt_i in range(DT):
                nc.sync.dma_start(xt[:, dt_i, 0:QC], xT_r[dt_i, :, 0:QC])
            for dt_i in range(DT):
                nc.sync.dma_start(wv_t[:, dt_i, :], wvT_r[dt_i, :, :])
                nc.sync.dma_start(wq_t[:, dt_i, :], wqT_r[dt_i, :, :])
            nc.sync.dma_start(wp_t[:, :, :], wpT2[:, :, :])
            for nh in range(1, NQC):
                for dt_i in range(DT):
                    nc.sync.dma_start(xt[:, dt_i, nh * QC:(nh + 1) * QC],
                                      xT_r[dt_i, :, nh * QC:(nh + 1) * QC])
            for j in range(NT):
                nc.vector.memset(vaug[:, j, :, 64], 1.0)

            # ---- background micro-step machinery ----
            bg = deque()

            def drain(k):
                done = 0
                while bg and done < k:
                    try:
                        next(bg[0])
                        done += 1
                    except StopIteration:
                        bg.popleft()

            # ---- QKV building blocks ----
            def qk_group_wide(w_t, dst, m, c):
                # [128, QC] group on the st tag (prefix only)
                kp = ps.tile([128, QC], FP32, tag="st", bufs=2, name="qkp")
                for dt_i in range(DT):
                    lhs = w_t[:, dt_i, m * 128:(m + 1) * 128]
                    for h2 in range(QC // 512):
                        nc.tensor.matmul(
                            kp[:, h2 * 512:(h2 + 1) * 512],
                            lhs,
                            xt[:, dt_i, c * QC + h2 * 512:
                               c * QC + (h2 + 1) * 512],
                            start=(dt_i == 0), stop=(dt_i == DT - 1),
                        )
                nc.vector.tensor_copy(dst[:, m, c * QC:(c + 1) * QC],
                                      kp[:, :])

            def qk_group_bg(w_t, dst, m, c5):
                # [128, 512] background group on the aux tag
                kp = ps.tile([128, 512], FP32, tag="aux", bufs=2, name="qkb")
                for dt_i in range(DT):
                    nc.tensor.matmul(
                        kp[:, :],
                        w_t[:, dt_i, m * 128:(m + 1) * 128],
                        xt[:, dt_i, c5 * 512:(c5 + 1) * 512],
                        start=(dt_i == 0), stop=(dt_i == DT - 1),
                    )
                    yield
                nc.vector.tensor_copy(dst[:, m, c5 * 512:(c5 + 1) * 512],
                                      kp[:, :])
                yield

            def v_group(j):
                vp = ps.tile([128, DG], FP32, tag="aux", bufs=2, name="vp")
                for dt_i in range(DT):
                    nc.tensor.matmul(
                        vp[:, :],
                        xt[:, dt_i, j * 128:(j + 1) * 128],
                        wv_t[:, dt_i, :],
                        start=(dt_i == 0), stop=(dt_i == DT - 1),
                    )
                for h in range(HL):
                    nc.vector.tensor_copy(vaug[:, j, h, 0:64],
                                          vp[:, h * 64:(h + 1) * 64])

            # ---- normalize + projection generators ----
            def norm_rest(osb, zrow, hh, c):
                pt, odd = hh // 2, hh % 2
                rz = wkpool.tile([1, 512], FP32, tag="rz", bufs=4, name="rz")
                nc.vector.reciprocal_approx_fast(rz[:, :], zrow[:, :])
                yield
                rzs = wkpool.tile([64, 512], FP32, tag="rzs", bufs=4,
                                  name="rzs")
                nc.gpsimd.partition_broadcast(rzs[:, :], rz[:, :])
                yield
                cs = slice(c * 512, (c + 1) * 512)
                if odd == 0:
                    nc.vector.tensor_tensor(otn[0:64, pt, cs],
                                            osb[0:64, :], rzs[:, :], MULT)
                    yield
                else:
                    ohst = wkpool.tile([64, 512], BF16, tag="ohst", bufs=4,
                                       name="ohst")
                    nc.vector.tensor_tensor(ohst[:, :], osb[0:64, :],
                                            rzs[:, :], MULT)
                    yield
                    nc.sync.dma_start(otn[64:128, pt, cs], ohst[:, :])
                    yield

            def proj_gen(c):
                for isub in range(4):
                    ib = c * 512 + isub * 128
                    for e in range(2):
                        pj = ps.tile([128, 512], FP32, tag="aux", bufs=2,
                                     name="pj")
                        for pt in range(2):
                            nc.tensor.matmul(
                                pj[:, :],
                                otn[:, pt, ib:ib + 128],
                                wp_t[:, pt, e * 512:(e + 1) * 512],
                                start=(pt == 0), stop=(pt == 1))
                            yield
                        ob = wkpool.tile([128, 512], FP32, tag="ob", bufs=3,
                                         name="ob")
                        nc.vector.tensor_copy(ob[:, :], pj[:, :])
                        nc.sync.dma_start(
                            out[ib:ib + 128, e * 512:(e + 1) * 512],
                            ob[:, :])
                        yield

            # ---- QKV prefix: K (all), V (all), Q (first QC cols both segs)
            for m in range(2):
                for c in range(NQC):
                    qk_group_wide(wk_t, kt, m, c)
            for j in range(NT):
                v_group(j)
            qk_group_wide(wq_t, qt, 0, 0)
            qk_group_wide(wq_t, qt, 1, 0)
            # remaining Q as background (512-wide groups)
            for c5 in range(QC // 512, NC):
                bg.append(qk_group_bg(wq_t, qt, 0, c5))
                bg.append(qk_group_bg(wq_t, qt, 1, c5))

            # ---- attention backbone ----
            for c in range(NC):
                for p in range(2):
                    he, ho = 2 * p, 2 * p + 1
                    ot_e = ps.tile([128, 512], FP32, tag="ot", bufs=2,
                                   name="ot_e")
                    ot_o = ps.tile([128, 512], FP32, tag="ot", bufs=2,
                                   name="ot_o")
                    for j in range(NT):
                        st = ps.tile([128, 1024], FP32, tag="st", bufs=2,
                                     name="st")
                        nc.tensor.matmul(
                            st[:, 0:512],
                            kt[0:64, p, j * 128:(j + 1) * 128],
                            qt[0:64, p, c * 512:(c + 1) * 512],
                            start=True, stop=True)
                        nc.tensor.matmul(
                            st[:, 512:1024],
                            kt[64:128, p, j * 128:(j + 1) * 128],
                            qt[64:128, p, c * 512:(c + 1) * 512],
                            start=True, stop=True)
                        et = sb.tile([128, 1024], BF16, tag="et", bufs=3,
                                     name="et")
                        nc.scalar.activation(et[:, :], st[:, :], EXP,
                                             scale=SCALE)
                        nc.tensor.matmul(
                            ot_e[0:65, :], vaug[:, j, he, 0:65],
                            et[:, 0:512],
                            start=(j == 0), stop=(j == NT - 1))
                        nc.tensor.matmul(
                            ot_o[0:65, :], vaug[:, j, ho, 0:65],
                            et[:, 512:1024],
                            start=(j == 0), stop=(j == NT - 1))
                        drain(1)
                    # free the PSUM accumulators with one copy each;
                    # the rest of the normalize chain runs in background
                    for hh, ot_h in ((he, ot_e), (ho, ot_o)):
                        osb = wkpool.tile([64, 512], BF16, tag="osb",
                                          bufs=8, name="osb")
                        nc.vector.tensor_copy(osb[:, :], ot_h[0:64, :])
                        zrow = wkpool.tile([1, 512], FP32, tag="zrow",
                                           bufs=8, name="zrow")
                        nc.vector.tensor_copy(zrow[:, :], ot_h[64:65, :])
                        ng = norm_rest(osb, zrow, hh, c)
                        if c == NC - 1 and p == 1:
                            for _ in ng:
                                pass
                        else:
                            bg.append(ng)
                pg = proj_gen(c)
                if c == NC - 1:
                    while bg:
                        drain(64)
                    for _ in pg:
                        pass
                else:
                    bg.append(pg)

            while bg:
                drain(64)

    nc.compile()
    return nc


_CACHED = {}


def _get_nc(n=N):
    if n not in _CACHED:
        _CACHED[n] = _build(n)
    return _CACHED[n]


def _make_in_maps(x, w_qkv, w_proj):
    import ml_dtypes
    bf16 = ml_dtypes.bfloat16
    in_maps = []
    for c in range(8):
        b, g = divmod(c, 4)
        s = slice(g * DG, (g + 1) * DG)
        wp = w_proj[:, s]  # [D(e), 256]
        in_maps.append({
            "xT": np.ascontiguousarray(x[b].T).astype(bf16),
            "wqT": np.ascontiguousarray(w_qkv[0 * D:1 * D][s, :].T).astype(bf16),
            "wkT": np.ascontiguousarray(w_qkv[1 * D:2 * D][s, :].T).astype(bf16),
            "wvT": np.ascontiguousarray(w_qkv[2 * D:3 * D][s, :].T).astype(bf16),
            "wpT2": np.ascontiguousarray(
                wp.T.reshape(2, 128, D).transpose(1, 0, 2)).astype(bf16),
        })
    return in_maps


def kernel(x, w_qkv, w_proj, b_proj):
    from concourse.bass_utils import run_bass_kernel_spmd

    nc = _get_nc(N)
    in_maps = _make_in_maps(np.asarray(x), np.asarray(w_qkv),
                            np.asarray(w_proj))
    res = run_bass_kernel_spmd(nc, in_maps, core_ids=list(range(8)))
    outs = [r["out"].astype(np.float32) for r in res.results]
    full = np.stack([outs[0] + outs[1] + outs[2] + outs[3],
                     outs[4] + outs[5] + outs[6] + outs[7]])
    full += np.asarray(b_proj, dtype=np.float32)[None, None, :]
    return full.astype(np.float32)


# revision 17
# speedup vs baseline: 1.0004x; 1.0004x over previous
"""Trainium2 Bass kernel for nn_Attention (B=2, N=4096, D=1024, 16 heads).

Sharding: 8 cores = 2 (batch) x 4 (head groups of 4 heads, Megatron TP).
Each core computes qkv for its 4 heads, flash-style attention (S^T layout,
softmax denominator via a ones-column folded into the V stationary), and its
partial output projection. The 4 partial projections per batch are summed on
the host during unshard (the TP all-reduce), plus the bias.

v2: software-pipelined instruction emission. The attention j-loop is the
backbone (ACT exp-bound, ~1.06us/iter); everything else (residual QKV
matmul groups, softmax normalization, output projection) is emitted as
background micro-steps interleaved into subsequent j-loops so the PE/ACT
streams never stall at block boundaries. PSUM accumulators are freed at
j-loop end by a single DVE copy to SBUF; the normalize chain (recip ->
gpsimd partition_broadcast -> multiply) runs off the critical path.
"""

from collections import deque

import numpy as np

import concourse.bacc as bacc
import concourse.mybir as mybir
import concourse.tile as tile

B = 2
N = 4096
D = 1024
HL = 4          # heads per core
HD = 64         # head dim
DG = HL * HD    # 256 = per-core d' width
SCALE = HD ** -0.5

FP32 = mybir.dt.float32
BF16 = mybir.dt.bfloat16
MULT = mybir.AluOpType.mult
EXP = mybir.ActivationFunctionType.Exp


def _build(n=N):
    nc = bacc.Bacc("TRN2", target_bir_lowering=False, debug=False)

    xT = nc.declare_dram_parameter("xT", [D, n], BF16, isOutput=False)
    wqT = nc.declare_dram_parameter("wqT", [D, DG], BF16, isOutput=False)
    wkT = nc.declare_dram_parameter("wkT", [D, DG], BF16, isOutput=False)
    wvT = nc.declare_dram_parameter("wvT", [D, DG], BF16, isOutput=False)
    wpT2 = nc.declare_dram_parameter("wpT2", [128, 2, D], BF16, isOutput=False)
    out = nc.declare_dram_parameter("out", [n, D], FP32, isOutput=True)

    DT = D // 128        # 8 contraction tiles for qkv
    NT = n // 128        # key tiles
    QC = min(1024, n)    # qkv prefix group width
    NQC = n // QC
    NC = n // 512        # attention i-chunks

    xT_r = xT.rearrange("(dt p) n -> dt p n", p=128)

    with tile.TileContext(nc) as tc:
        with (
            tc.tile_pool(name="sb", bufs=1) as sb,
            tc.tile_pool(name="wkp", bufs=1) as wkpool,
            tc.tile_pool(name="ps", bufs=1, space="PSUM") as ps,
        ):
            # ---- persistent SBUF tiles ----
            xt = sb.tile([128, DT, n], BF16, tag="xt")
            wq_t = sb.tile([128, DT, DG], BF16, tag="wq")
            wk_t = sb.tile([128, DT, DG], BF16, tag="wk")
            wv_t = sb.tile([128, DT, DG], BF16, tag="wv")
            wp_t = sb.tile([128, 2, D], BF16, tag="wp")
            qt = sb.tile([128, 2, n], BF16, tag="qt")
            kt = sb.tile([128, 2, n], BF16, tag="kt")
            vaug = sb.tile([128, NT, HL, 65], BF16, tag="vaug")
            otn = sb.tile([128, 2, n], BF16, tag="otn")

            # ---- load weights + x^T ----
            wqT_r = wqT.rearrange("(dt p) m -> dt p m", p=128)
            wkT_r = wkT.rearrange("(dt p) m -> dt p m", p=128)
            wvT_r = wvT.rearrange("(dt p) m -> dt p m", p=128)
            for dt_i in range(DT):
                nc.sync.dma_start(wk_t[:, dt_i, :], wkT_r[dt_i, :, :])
            for dt_i in range(DT):
                nc.sync.dma_start(xt[:, dt_i, 0:QC], xT_r[dt_i, :, 0:QC])
            for dt_i in range(DT):
                nc.sync.dma_start(wv_t[:, dt_i, :], wvT_r[dt_i, :, :])
                nc.sync.dma_start(wq_t[:, dt_i, :], wqT_r[dt_i, :, :])
            nc.sync.dma_start(wp_t[:, :, :], wpT2[:, :, :])
            for nh in range(1, NQC):
                for dt_i in range(DT):
                    nc.sync.dma_start(xt[:, dt_i, nh * QC:(nh + 1) * QC],
                                      xT_r[dt_i, :, nh * QC:(nh + 1) * QC])
            for j in range(NT):
                nc.vector.memset(vaug[:, j, :, 64], 1.0)

            # ---- background micro-step machinery ----
            bg = deque()

            def drain(k):
                done = 0
                while bg and done < k:
                    try:
                        next(bg[0])
                        done += 1
                    except StopIteration:
                        bg.popleft()

            # ---- QKV building blocks ----
            def qk_group_wide(w_t, dst, m, c):
                # [128, QC] group on the st tag (prefix only)
                kp = ps.tile([128, QC], FP32, tag="st", bufs=2, name="qkp")
                for dt_i in range(DT):
                    lhs = w_t[:, dt_i, m * 128:(m + 1) * 128]
                    for h2 in range(QC // 512):
                        nc.tensor.matmul(
                            kp[:, h2 * 512:(h2 + 1) * 512],
                            lhs,
                            xt[:, dt_i, c * QC + h2 * 512:
                               c * QC + (h2 + 1) * 512],
                            start=(dt_i == 0), stop=(dt_i == DT - 1),
                        )
                nc.vector.tensor_copy(dst[:, m, c * QC:(c + 1) * QC],
                                      kp[:, :])

            def qk_group_bg(w_t, dst, m, c5):
                # [128, 512] background group on the aux tag
                kp = ps.tile([128, 512], FP32, tag="aux", bufs=2, name="qkb")
                for dt_i in range(DT):
                    nc.tensor.matmul(
                        kp[:, :],
                        w_t[:, dt_i, m * 128:(m + 1) * 128],
                        xt[:, dt_i, c5 * 512:(c5 + 1) * 512],
                        start=(dt_i == 0), stop=(dt_i == DT - 1),
                    )
                    yield
                nc.vector.tensor_copy(dst[:, m, c5 * 512:(c5 + 1) * 512],
                                      kp[:, :])
                yield

            def v_group(j):
                vp = ps.tile([128, DG], FP32, tag="aux", bufs=2, name="vp")
                for dt_i in range(DT):
                    nc.tensor.matmul(
                        vp[:, :],
                        xt[:, dt_i, j * 128:(j + 1) * 128],
                        wv_t[:, dt_i, :],
                        start=(dt_i == 0), stop=(dt_i == DT - 1),
                    )
                for h in range(HL):
                    nc.vector.tensor_copy(vaug[:, j, h, 0:64],
                                          vp[:, h * 64:(h + 1) * 64])

            # ---- normalize + projection generators ----
            def norm_rest(osb, zrow, hh, c):
                pt, odd = hh // 2, hh % 2
                rz = wkpool.tile([1, 512], FP32, tag="rz", bufs=4, name="rz")
                nc.vector.reciprocal_approx_fast(rz[:, :], zrow[:, :])
                yield
                rzs = wkpool.tile([64, 512], FP32, tag="rzs", bufs=4,
                                  name="rzs")
                nc.gpsimd.partition_broadcast(rzs[:, :], rz[:, :])
                yield
                cs = slice(c * 512, (c + 1) * 512)
                if odd == 0:
                    nc.vector.tensor_tensor(otn[0:64, pt, cs],
                                            osb[0:64, :], rzs[:, :], MULT)
                    yield
                else:
                    ohst = wkpool.tile([64, 512], BF16, tag="ohst", bufs=4,
                                       name="ohst")
                    nc.vector.tensor_tensor(ohst[:, :], osb[0:64, :],
                                            rzs[:, :], MULT)
                    yield
                    nc.sync.dma_start(otn[64:128, pt, cs], ohst[:, :])
                    yield

            def proj_gen(c):
                for isub in range(4):
                    ib = c * 512 + isub * 128
                    for e in range(2):
                        pj = ps.tile([128, 512], FP32, tag="aux", bufs=2,
                                     name="pj")
                        for pt in range(2):
                            nc.tensor.matmul(
                                pj[:, :],
                                otn[:, pt, ib:ib + 128],
                                wp_t[:, pt, e * 512:(e + 1) * 512],
                                start=(pt == 0), stop=(pt == 1))
                            yield
                        ob = wkpool.tile([128, 512], FP32, tag="ob", bufs=3,
                                         name="ob")
                        nc.vector.tensor_copy(ob[:, :], pj[:, :])
                        nc.sync.dma_start(
                            out[ib:ib + 128, e * 512:(e + 1) * 512],
                            ob[:, :])
                        yield

            # ---- QKV prefix: K (all), V (all), Q (first QC cols both segs)
            for m in range(2):
                for c in range(NQC):
                    qk_group_wide(wk_t, kt, m, c)
            for j in range(NT):
                v_group(j)
            qk_group_wide(wq_t, qt, 0, 0)
            qk_group_wide(wq_t, qt, 1, 0)
            # remaining Q as background (512-wide groups)
            for c5 in range(QC // 512, NC):
                bg.append(qk_group_bg(wq_t, qt, 0, c5))
                bg.append(qk_group_bg(wq_t, qt, 1, c5))

            # ---- attention backbone ----
            for c in range(NC):
                for p in range(2):
                    he, ho = 2 * p, 2 * p + 1
                    ot_e = ps.tile([128, 512], FP32, tag="ot", bufs=2,
                                   name="ot_e")
                    ot_o = ps.tile([128, 512], FP32, tag="ot", bufs=2,
                                   name="ot_o")
                    pend = None
                    for j in range(NT):
                        st = ps.tile([128, 1024], FP32, tag="st", bufs=2,
                                     name="st")
                        nc.tensor.matmul(
                            st[:, 0:512],
                            kt[0:64, p, j * 128:(j + 1) * 128],
                            qt[0:64, p, c * 512:(c + 1) * 512],
                            start=True, stop=True)
                        nc.tensor.matmul(
                            st[:, 512:1024],
                            kt[64:128, p, j * 128:(j + 1) * 128],
                            qt[64:128, p, c * 512:(c + 1) * 512],
                            start=True, stop=True)
                        et = sb.tile([128, 1024], BF16, tag="et", bufs=3,
                                     name="et")
                        nc.scalar.activation(et[:, :], st[:, :], EXP,
                                             scale=SCALE)
                        if pend is not None:
                            pj_, pet = pend
                            nc.tensor.matmul(
                                ot_e[0:65, :], vaug[:, pj_, he, 0:65],
                                pet[:, 0:512],
                                start=(pj_ == 0), stop=False)
                            nc.tensor.matmul(
                                ot_o[0:65, :], vaug[:, pj_, ho, 0:65],
                                pet[:, 512:1024],
                                start=(pj_ == 0), stop=False)
                        pend = (j, et)
                        drain(1)
                    pj_, pet = pend
                    nc.tensor.matmul(
                        ot_e[0:65, :], vaug[:, pj_, he, 0:65],
                        pet[:, 0:512], start=False, stop=True)
                    nc.tensor.matmul(
                        ot_o[0:65, :], vaug[:, pj_, ho, 0:65],
                        pet[:, 512:1024], start=False, stop=True)
                    # free the PSUM accumulators with one copy each;
                    # the rest of the normalize chain runs in background
                    for hh, ot_h in ((he, ot_e), (ho, ot_o)):
                        osb = wkpool.tile([64, 512], BF16, tag="osb",
                                          bufs=8, name="osb")
                        nc.vector.tensor_copy(osb[:, :], ot_h[0:64, :])
                        zrow = wkpool.tile([1, 512], FP32, tag="zrow",
                                           bufs=8, name="zrow")
                        nc.vector.tensor_copy(zrow[:, :], ot_h[64:65, :])
                        ng = norm_rest(osb, zrow, hh, c)
                        if c == NC - 1 and p == 1:
                            for _ in ng:
                                pass
                        else:
                            bg.append(ng)
                pg = proj_gen(c)
                if c == NC - 1:
                    while bg:
                        drain(64)
                    for _ in pg:
                        pass
                else:
                    bg.append(pg)

            while bg:
                drain(64)

    nc.compile()
    return nc


_CACHED = {}


def _get_nc(n=N):
    if n not in _CACHED:
        _CACHED[n] = _build(n)
    return _CACHED[n]


def _make_in_maps(x, w_qkv, w_proj):
    import ml_dtypes
    bf16 = ml_dtypes.bfloat16
    in_maps = []
    for c in range(8):
        b, g = divmod(c, 4)
        s = slice(g * DG, (g + 1) * DG)
        wp = w_proj[:, s]  # [D(e), 256]
        in_maps.append({
            "xT": np.ascontiguousarray(x[b].T).astype(bf16),
            "wqT": np.ascontiguousarray(w_qkv[0 * D:1 * D][s, :].T).astype(bf16),
            "wkT": np.ascontiguousarray(w_qkv[1 * D:2 * D][s, :].T).astype(bf16),
            "wvT": np.ascontiguousarray(w_qkv[2 * D:3 * D][s, :].T).astype(bf16),
            "wpT2": np.ascontiguousarray(
                wp.T.reshape(2, 128, D).transpose(1, 0, 2)).astype(bf16),
        })
    return in_maps


def kernel(x, w_qkv, w_proj, b_proj):
    from concourse.bass_utils import run_bass_kernel_spmd

    nc = _get_nc(N)
    in_maps = _make_in_maps(np.asarray(x), np.asarray(w_qkv),
                            np.asarray(w_proj))
    res = run_bass_kernel_spmd(nc, in_maps, core_ids=list(range(8)))
    outs = [r["out"].astype(np.float32) for r in res.results]
    full = np.stack([outs[0] + outs[1] + outs[2] + outs[3],
                     outs[4] + outs[5] + outs[6] + outs[7]])
    full += np.asarray(b_proj, dtype=np.float32)[None, None, :]
    return full.astype(np.float32)


# revision 20
# speedup vs baseline: 1.0022x; 1.0018x over previous
"""Trainium2 Bass kernel for nn_Attention (B=2, N=4096, D=1024, 16 heads).

Sharding: 8 cores = 2 (batch) x 4 (head groups of 4 heads, Megatron TP).
Each core computes qkv for its 4 heads, flash-style attention (S^T layout,
softmax denominator via a ones-column folded into the V stationary), and its
partial output projection. The 4 partial projections per batch are summed on
the host during unshard (the TP all-reduce), plus the bias.

v2: software-pipelined instruction emission. The attention j-loop is the
backbone (ACT exp-bound, ~1.06us/iter); everything else (residual QKV
matmul groups, softmax normalization, output projection) is emitted as
background micro-steps interleaved into subsequent j-loops so the PE/ACT
streams never stall at block boundaries. PSUM accumulators are freed at
j-loop end by a single DVE copy to SBUF; the normalize chain (recip ->
gpsimd partition_broadcast -> multiply) runs off the critical path.
"""

from collections import deque

import numpy as np

import concourse.bacc as bacc
import concourse.mybir as mybir
import concourse.tile as tile

B = 2
N = 4096
D = 1024
HL = 4          # heads per core
HD = 64         # head dim
DG = HL * HD    # 256 = per-core d' width
SCALE = HD ** -0.5

FP32 = mybir.dt.float32
BF16 = mybir.dt.bfloat16
MULT = mybir.AluOpType.mult
EXP = mybir.ActivationFunctionType.Exp


def _build(n=N):
    nc = bacc.Bacc("TRN2", target_bir_lowering=False, debug=False)

    xT = nc.declare_dram_parameter("xT", [D, n], BF16, isOutput=False)
    wqT = nc.declare_dram_parameter("wqT", [D, DG], BF16, isOutput=False)
    wkT = nc.declare_dram_parameter("wkT", [D, DG], BF16, isOutput=False)
    wvT = nc.declare_dram_parameter("wvT", [D, DG], BF16, isOutput=False)
    wpT2 = nc.declare_dram_parameter("wpT2", [128, 2, D], BF16, isOutput=False)
    out = nc.declare_dram_parameter("out", [n, D], FP32, isOutput=True)

    DT = D // 128        # 8 contraction tiles for qkv
    NT = n // 128        # key tiles
    QC = min(1024, n)    # qkv prefix group width
    NQC = n // QC
    NC = n // 512        # attention i-chunks

    xT_r = xT.rearrange("(dt p) n -> dt p n", p=128)

    with tile.TileContext(nc) as tc:
        with (
            tc.tile_pool(name="sb", bufs=1) as sb,
            tc.tile_pool(name="wkp", bufs=1) as wkpool,
            tc.tile_pool(name="ps", bufs=1, space="PSUM") as ps,
        ):
            # ---- persistent SBUF tiles ----
            xt = sb.tile([128, DT, n], BF16, tag="xt")
            wq_t = sb.tile([128, DT, DG], BF16, tag="wq")
            wk_t = sb.tile([128, DT, DG], BF16, tag="wk")
            wv_t = sb.tile([128, DT, DG], BF16, tag="wv")
            wp_t = sb.tile([128, 2, D], BF16, tag="wp")
            qt = sb.tile([128, 2, n], BF16, tag="qt")
            kt = sb.tile([128, 2, n], BF16, tag="kt")
            vaug = sb.tile([128, NT, HL, 65], BF16, tag="vaug")
            otn = sb.tile([128, 2, n], BF16, tag="otn")

            # ---- load weights + x^T ----
            wqT_r = wqT.rearrange("(dt p) m -> dt p m", p=128)
            wkT_r = wkT.rearrange("(dt p) m -> dt p m", p=128)
            wvT_r = wvT.rearrange("(dt p) m -> dt p m", p=128)
            for dt_i in range(DT):
                nc.sync.dma_start(wk_t[:, dt_i, :], wkT_r[dt_i, :, :])
            for dt_i in range(DT):
                nc.sync.dma_start(xt[:, dt_i, 0:QC], xT_r[dt_i, :, 0:QC])
            for dt_i in range(DT):
                nc.sync.dma_start(wv_t[:, dt_i, :], wvT_r[dt_i, :, :])
                nc.sync.dma_start(wq_t[:, dt_i, :], wqT_r[dt_i, :, :])
            nc.sync.dma_start(wp_t[:, :, :], wpT2[:, :, :])
            for nh in range(1, NQC):
                for dt_i in range(DT):
                    nc.sync.dma_start(xt[:, dt_i, nh * QC:(nh + 1) * QC],
                                      xT_r[dt_i, :, nh * QC:(nh + 1) * QC])
            for j in range(NT):
                nc.vector.memset(vaug[:, j, :, 64], 1.0)

            # ---- background micro-step machinery ----
            bg = deque()

            def drain(k):
                done = 0
                while bg and done < k:
                    try:
                        next(bg[0])
                        done += 1
                    except StopIteration:
                        bg.popleft()

            # ---- QKV building blocks ----
            def qk_group_wide(w_t, dst, m, c):
                # [128, QC] group on the st tag (prefix only)
                kp = ps.tile([128, QC], FP32, tag="st", bufs=2, name="qkp")
                for dt_i in range(DT):
                    lhs = w_t[:, dt_i, m * 128:(m + 1) * 128]
                    for h2 in range(QC // 512):
                        nc.tensor.matmul(
                            kp[:, h2 * 512:(h2 + 1) * 512],
                            lhs,
                            xt[:, dt_i, c * QC + h2 * 512:
                               c * QC + (h2 + 1) * 512],
                            start=(dt_i == 0), stop=(dt_i == DT - 1),
                        )
                nc.vector.tensor_copy(dst[:, m, c * QC:(c + 1) * QC],
                                      kp[:, :])

            def qk_group_bg(w_t, dst, m, c5):
                # [128, 512] background group on the aux tag
                kp = ps.tile([128, 512], FP32, tag="aux", bufs=2, name="qkb")
                for dt_i in range(DT):
                    nc.tensor.matmul(
                        kp[:, :],
                        w_t[:, dt_i, m * 128:(m + 1) * 128],
                        xt[:, dt_i, c5 * 512:(c5 + 1) * 512],
                        start=(dt_i == 0), stop=(dt_i == DT - 1),
                    )
                    yield
                nc.vector.tensor_copy(dst[:, m, c5 * 512:(c5 + 1) * 512],
                                      kp[:, :])
                yield

            def v_group(j):
                vp = ps.tile([128, DG], FP32, tag="aux", bufs=2, name="vp")
                for dt_i in range(DT):
                    nc.tensor.matmul(
                        vp[:, :],
                        xt[:, dt_i, j * 128:(j + 1) * 128],
                        wv_t[:, dt_i, :],
                        start=(dt_i == 0), stop=(dt_i == DT - 1),
                    )
                for h in range(HL):
                    nc.vector.tensor_copy(vaug[:, j, h, 0:64],
                                          vp[:, h * 64:(h + 1) * 64])

            # ---- normalize + projection generators ----
            def norm_rest(osb, zrow, hh, c):
                pt, odd = hh // 2, hh % 2
                rz = wkpool.tile([1, 512], FP32, tag="rz", bufs=4, name="rz")
                nc.vector.reciprocal_approx_fast(rz[:, :], zrow[:, :])
                yield
                rzs = wkpool.tile([64, 512], FP32, tag="rzs", bufs=4,
                                  name="rzs")
                nc.gpsimd.partition_broadcast(rzs[:, :], rz[:, :])
                yield
                cs = slice(c * 512, (c + 1) * 512)
                if odd == 0:
                    nc.vector.tensor_tensor(otn[0:64, pt, cs],
                                            osb[0:64, :], rzs[:, :], MULT)
                    yield
                else:
                    ohst = wkpool.tile([64, 512], BF16, tag="ohst", bufs=4,
                                       name="ohst")
                    nc.vector.tensor_tensor(ohst[:, :], osb[0:64, :],
                                            rzs[:, :], MULT)
                    yield
                    nc.sync.dma_start(otn[64:128, pt, cs], ohst[:, :])
                    yield

            def proj_gen(c):
                for isub in range(4):
                    ib = c * 512 + isub * 128
                    for e in range(2):
                        pj = ps.tile([128, 512], FP32, tag="aux", bufs=2,
                                     name="pj")
                        for pt in range(2):
                            nc.tensor.matmul(
                                pj[:, :],
                                otn[:, pt, ib:ib + 128],
                                wp_t[:, pt, e * 512:(e + 1) * 512],
                                start=(pt == 0), stop=(pt == 1))
                            yield
                        ob = wkpool.tile([128, 512], FP32, tag="ob", bufs=3,
                                         name="ob")
                        nc.vector.tensor_copy(ob[:, :], pj[:, :])
                        nc.sync.dma_start(
                            out[ib:ib + 128, e * 512:(e + 1) * 512],
                            ob[:, :])
                        yield

            # ---- QKV prefix: K (all), V (all), Q (first QC cols both segs)
            for m in range(2):
                for c in range(NQC):
                    qk_group_wide(wk_t, kt, m, c)
            for j in range(NT):
                v_group(j)
            qk_group_wide(wq_t, qt, 0, 0)
            qk_group_wide(wq_t, qt, 1, 0)
            # remaining Q as background (512-wide groups)
            for c5 in range(QC // 512, NC):
                bg.append(qk_group_bg(wq_t, qt, 0, c5))
                bg.append(qk_group_bg(wq_t, qt, 1, c5))

            # ---- attention backbone ----
            for c in range(NC):
                for p in range(2):
                    he, ho = 2 * p, 2 * p + 1
                    ot_e = ps.tile([128, 512], FP32, tag="ot", bufs=2,
                                   name="ot_e")
                    ot_o = ps.tile([128, 512], FP32, tag="ot", bufs=2,
                                   name="ot_o")
                    pend = None
                    for j in range(NT):
                        st = ps.tile([128, 1024], FP32, tag="st", bufs=2,
                                     name="st")
                        nc.tensor.matmul(
                            st[:, 0:512],
                            kt[0:64, p, j * 128:(j + 1) * 128],
                            qt[0:64, p, c * 512:(c + 1) * 512],
                            start=True, stop=True)
                        nc.tensor.matmul(
                            st[:, 512:1024],
                            kt[64:128, p, j * 128:(j + 1) * 128],
                            qt[64:128, p, c * 512:(c + 1) * 512],
                            start=True, stop=True)
                        et = sb.tile([128, 1024], BF16, tag="et", bufs=3,
                                     name="et", padded_shape=[128, 2048])
                        nc.scalar.activation(et[:, :], st[:, :], EXP,
                                             scale=SCALE)
                        if pend is not None:
                            pj_, pet = pend
                            nc.tensor.matmul(
                                ot_e[0:65, :], vaug[:, pj_, he, 0:65],
                                pet[:, 0:512],
                                start=(pj_ == 0), stop=False)
                            nc.tensor.matmul(
                                ot_o[0:65, :], vaug[:, pj_, ho, 0:65],
                                pet[:, 512:1024],
                                start=(pj_ == 0), stop=False)
                        pend = (j, et)
                        drain(1)
                    pj_, pet = pend
                    nc.tensor.matmul(
                        ot_e[0:65, :], vaug[:, pj_, he, 0:65],
                        pet[:, 0:512], start=False, stop=True)
                    nc.tensor.matmul(
                        ot_o[0:65, :], vaug[:, pj_, ho, 0:65],
                        pet[:, 512:1024], start=False, stop=True)
                    # free the PSUM accumulators with one copy each;
                    # the rest of the normalize chain runs in background
                    for hh, ot_h in ((he, ot_e), (ho, ot_o)):
                        osb = wkpool.tile([64, 512], BF16, tag="osb",
                                          bufs=8, name="osb")
                        nc.vector.tensor_copy(osb[:, :], ot_h[0:64, :])
                        zrow = wkpool.tile([1, 512], FP32, tag="zrow",
                                           bufs=8, name="zrow")
                        nc.vector.tensor_copy(zrow[:, :], ot_h[64:65, :])
                        ng = norm_rest(osb, zrow, hh, c)
                        if c == NC - 1 and p == 1:
                            for _ in ng:
                                pass
                        else:
                            bg.append(ng)
                pg = proj_gen(c)
                if c == NC - 1:
                    while bg:
                        drain(64)
                    for _ in pg:
                        pass
                else:
                    bg.append(pg)

            while bg:
                drain(64)

    nc.compile()
    return nc


_CACHED = {}


def _get_nc(n=N):
    if n not in _CACHED:
        _CACHED[n] = _build(n)
    return _CACHED[n]


def _make_in_maps(x, w_qkv, w_proj):
    import ml_dtypes
    bf16 = ml_dtypes.bfloat16
    in_maps = []
    for c in range(8):
        b, g = divmod(c, 4)
        s = slice(g * DG, (g + 1) * DG)
        wp = w_proj[:, s]  # [D(e), 256]
        in_maps.append({
            "xT": np.ascontiguousarray(x[b].T).astype(bf16),
            "wqT": np.ascontiguousarray(w_qkv[0 * D:1 * D][s, :].T).astype(bf16),
            "wkT": np.ascontiguousarray(w_qkv[1 * D:2 * D][s, :].T).astype(bf16),
            "wvT": np.ascontiguousarray(w_qkv[2 * D:3 * D][s, :].T).astype(bf16),
            "wpT2": np.ascontiguousarray(
                wp.T.reshape(2, 128, D).transpose(1, 0, 2)).astype(bf16),
        })
    return in_maps


def kernel(x, w_qkv, w_proj, b_proj):
    from concourse.bass_utils import run_bass_kernel_spmd

    nc = _get_nc(N)
    in_maps = _make_in_maps(np.asarray(x), np.asarray(w_qkv),
                            np.asarray(w_proj))
    res = run_bass_kernel_spmd(nc, in_maps, core_ids=list(range(8)))
    outs = [r["out"].astype(np.float32) for r in res.results]
    full = np.stack([outs[0] + outs[1] + outs[2] + outs[3],
                     outs[4] + outs[5] + outs[6] + outs[7]])
    full += np.asarray(b_proj, dtype=np.float32)[None, None, :]
    return full.astype(np.float32)
